# revision 2
# baseline (speedup 1.0000x reference)
"""Trainium2 Bass kernel for nn_Euclidean (retrieval_knn) — V5.

out[b, c] = -mean_f (x[b,f] - w[c,f])^2
          = (2/F)*(x @ w.T) - ||x_b||^2/F - ||w_c||^2/F

Sharding: data-parallel over batch across 8 NeuronCores; w replicated.

Per-core dataflow (computes outT[c, b] on device, bf16 store, host widens):
  - Host stages xT bf16 span tiles, wT fp8e4 (c-padded to 1024), row bf16 w.
  - DMA loads ride ONE ring in consumption order (span0, wT, w rows 0-3,
    span1, w rows 4-7, span2, span3); stores ride the scalar-engine ring.
  - Main loop is b-span-outer / c-chunk-inner with one PSUM bank per c-chunk,
    so matmuls start as soon as span0 + wT land. fp8 DoubleRow matmuls (the
    per-matmul LDWEIGHTS is hidden by the PE reorder window).
  - ||w_c||^2: ScalarE Square+accum on row-major w -> f32 per-partition bias
    at evacuation (mean-center constant -1.0 folded in).
  - ||x_b||^2 entirely off the PE: GpSimd squares, DVE strided f-chunk
    reduce, GpSimd partition_all_reduce, GpSimd scale/offset -> bf16 row,
    injected into PSUM by one K=1 matmul per (span, c-chunk).
"""

import math
import os
import sys

import ml_dtypes
import numpy as np

if "/opt/trn_rl_repo" not in sys.path:
    sys.path.insert(0, "/opt/trn_rl_repo")

N_CORES = 8
B_TOTAL = 16384
F = 2048
C = 1000

P = 128
B = B_TOTAL // N_CORES   # 2048 rows per core
KT = F // P              # 16 k-chunks
KP = KT // 2             # 8 DoubleRow k-pairs
NSPAN = 4                # b-spans of 512
SPAN = B // NSPAN
CT = 8                   # c-chunks (7x128 + 104)
CPAD = 1024

_cache = {}
LAST_RESULTS = None


def _legalize_waits(nc):
    """Walrus encodes at most ONE sync-wait per instruction struct, but Tile's
    sem assignment freely attaches several. Split: hoist all but the last wait
    onto standalone EventSemaphore instructions (pure sem-op carriers) placed
    immediately before the over-limit instruction on the same engine queue."""
    import bass_rust
    import concourse.mybir as mybir

    n = 0
    for f in nc.m.functions:
        for bb in f.blocks:
            newlist = []
            for inst in bb.instructions:
                si = inst.sync_info
                if si is not None and len(si.on_wait) > 1:
                    waits = list(si.on_wait)
                    for w in waits[:-1]:
                        ev = mybir.InstEventSemaphore(
                            name=f"waitsplit_{n}", ins=[], outs=[]
                        )
                        ev.engine = inst.engine
                        ev.sync_info = bass_rust.SyncInfo(on_wait=[w], on_update=[])
                        newlist.append(ev)
                        n += 1
                    inst.sync_info = bass_rust.SyncInfo(
                        on_wait=[waits[-1]], on_update=list(si.on_update)
                    )
                newlist.append(inst)
            bb.instructions = newlist
    return n


def _build():
    import concourse.bass as bass
    import concourse.bass_isa as bass_isa
    import concourse.mybir as mybir
    from concourse.tile import TileContext

    bdt = mybir.dt.bfloat16
    fdt = mybir.dt.float32
    f8 = mybir.dt.float8e4
    AF = mybir.ActivationFunctionType
    ALU = mybir.AluOpType
    DR = mybir.MatmulPerfMode.DoubleRow

    nc = bass.Bass()
    # xt[s*128+p, fc, b] = x_shard[s*512+b, fc*128+p]   (bf16)
    xt = nc.dram_tensor("xt", [NSPAN * P, KT, SPAN], bdt, kind="ExternalInput")
    # wt[p, fc, c] = w[c, fc*128+p]                      (fp8, c-padded)
    wt = nc.dram_tensor("wt", [P, KT, CPAD], f8, kind="ExternalInput")
    # row-major bf16 w for the norm pass
    wr = nc.dram_tensor("wr", [C, F], bdt, kind="ExternalInput")
    oT = nc.dram_tensor("outT", [C, B], bdt, kind="ExternalOutput")

    inv_sqrt_f = 1.0 / math.sqrt(F)

    with TileContext(nc) as tc:
        with (
            tc.tile_pool(name="consts", bufs=1) as constp,
            tc.tile_pool(name="xstage", bufs=2) as xp,
            tc.tile_pool(name="wstage", bufs=2) as wp,
            tc.tile_pool(name="evac", bufs=4) as ep,
            tc.tile_pool(name="psum", bufs=1, space="PSUM") as pp,
        ):
            ones_row = constp.tile([1, P], bdt)   # x2-inject stationary
            nc.vector.memset(ones_row[:, :], 1.0)
            ones_col = constp.tile([P, 1], bdt)   # norm-matmul stationary
            nc.vector.memset(ones_col[:, :], 1.0)
            neg1 = constp.tile([P, 1], fdt)       # -1.0 bias (mean-center)
            nc.vector.memset(neg1[:, :], -1.0)
            negx2row = constp.tile([1, B], bdt)   # -(||x_b||^2 - 2048)/2
            negw2 = constp.tile([P, CT], fdt)     # -||w_c||^2/F - 1.0
            wt_sb = constp.tile([P, KT, CPAD], f8)
            xf8 = constp.tile([P, KT, B], f8)

            # ---- DMA issue order on the sync ring == consumption order.
            xts_tiles = []
            wrow_tiles = []

            def load_span(s):
                t = xp.tile([P, KT, SPAN], bdt, name="xts", tag="xts", bufs=3)
                nc.sync.dma_start(
                    out=t[:, :, :], in_=xt[s * P : (s + 1) * P, :, :]
                )
                xts_tiles.append(t)

            def load_wrow(cc):
                c0 = cc * P
                csz = min(P, C - c0)
                t = wp.tile([P, F], bdt, name="wrow", tag="wrow", bufs=8)
                nc.sync.dma_start(out=t[:csz, :], in_=wr[c0 : c0 + csz, :])
                wrow_tiles.append(t)

            load_span(0)
            nc.sync.dma_start(out=wt_sb[:, :, :], in_=wt[:, :, :])
            for cc in range(4):
                load_wrow(cc)
            load_span(1)
            for cc in range(4, CT):
                load_wrow(cc)
            load_span(2)
            load_span(3)

            # ---- w norms on ScalarE: first half before the main loop, the
            # second half interleaved so bs0's early evacuations aren't
            # queued behind squares whose DMA lands late.
            def w_setup(cc):
                c0 = cc * P
                csz = min(P, C - c0)
                wsq = wp.tile([P, F], bdt, tag="wsq", bufs=2)
                w2c = wp.tile([P, 1], fdt, tag="w2c", bufs=2)
                nc.scalar.activation(
                    wsq[:csz, :], wrow_tiles[cc][:csz, :], AF.Square,
                    scale=inv_sqrt_f, accum_out=w2c[:csz, :],
                )
                nc.scalar.activation(
                    negw2[:csz, cc : cc + 1], w2c[:csz, :], AF.Identity,
                    scale=-1.0, bias=neg1[:csz, :],
                )

            for cc in range(4):
                w_setup(cc)

            # ---- merged per-span pipeline: DVE cast+square, PE ones-matmul
            # norm reduction, DVE rescale, then the span's c-chunk matmul
            # pass. Per-engine queue order matters: nx_s must precede
            # cast_{s+1} on DVE, norm-MMs_s must sit between bs passes on PE.
            # PSUM = 4 main banks (c-chunks mod 4) + 2 norm banks.
            for bs in range(NSPAN):
                xts = xts_tiles[bs]
                nc.vector.tensor_copy(
                    xf8[:, :, bs * SPAN : (bs + 1) * SPAN], xts[:, :, :]
                )
                xsq = xp.tile([P, KT, SPAN], bdt, tag="xsq", bufs=2)
                nc.vector.tensor_mul(xsq[:, :, :], xts[:, :, :], xts[:, :, :])
                # ||x_b||^2: 16 ones-matmuls reduce xsq over partitions and
                # f-chunks; DVE rescales to the mean-centered bf16 row
                # consumed by the K=1 inject matmuls.
                x2ps = pp.tile([1, SPAN], fdt, name="x2ps", tag="x2ps", bufs=2)
                for fc in range(KT):
                    nc.tensor.matmul(
                        x2ps[:, :], ones_col[:, :], xsq[:, fc, :],
                        start=(fc == 0), stop=(fc == KT - 1),
                    )
                nc.vector.tensor_scalar(
                    negx2row[0:1, bs * SPAN : (bs + 1) * SPAN], x2ps[0:1, :],
                    -0.5, float(F) / 2.0, op0=ALU.mult, op1=ALU.add,
                )
                for cc in range(CT):
                    c0 = cc * P
                    csz = min(P, C - c0)
                    ps = pp.tile(
                        [P, SPAN], fdt, name=f"mm{cc % 4}", tag=f"mm{cc % 4}",
                        bufs=1,
                    )
                    for kp in range(KP):
                        nc.tensor.matmul(
                            ps[:csz, :],
                            wt_sb[:, 2 * kp : 2 * kp + 2, c0 : c0 + csz],
                            xf8[:, 2 * kp : 2 * kp + 2,
                                bs * SPAN : (bs + 1) * SPAN],
                            start=(kp == 0), stop=False,
                            perf_mode=DR,
                        )
                    nc.tensor.matmul(
                        ps[:csz, :],
                        ones_row[0:1, 0:csz],
                        negx2row[0:1, bs * SPAN : (bs + 1) * SPAN],
                        start=False, stop=True,
                    )
                    osb = ep.tile([P, SPAN], bdt, tag="osb", bufs=4)
                    nc.scalar.activation(
                        osb[:csz, :], ps[:csz, :], AF.Identity,
                        scale=2.0 / F, bias=negw2[:csz, cc : cc + 1],
                    )
                    nc.scalar.dma_start(
                        out=oT[c0 : c0 + csz, bs * SPAN : (bs + 1) * SPAN],
                        in_=osb[:csz, :],
                    )
                    if bs == 0 and cc == 3:
                        for wcc in range(4, CT):
                            w_setup(wcc)

    return nc


def _stage_inputs(x, w):
    """Host-side layout/dtype staging. Returns per-core input maps."""
    bf16 = ml_dtypes.bfloat16
    f8 = ml_dtypes.float8_e4m3

    wt_host = np.zeros((P, KT, CPAD), dtype=f8)
    wt_host[:, :, :C] = (
        w.T.reshape(KT, P, C).transpose(1, 0, 2).astype(f8)
    )
    wr_host = w.astype(bf16)

    in_maps = []
    for i in range(N_CORES):
        xs = x[i * B : (i + 1) * B]                    # [B, F] f32
        a = xs.T.reshape(KT, P, NSPAN, SPAN)           # [fc, p, s, b]
        xt_host = np.ascontiguousarray(a.transpose(2, 1, 0, 3)).astype(bf16)
        in_maps.append(
            {
                "xt": xt_host.reshape(NSPAN * P, KT, SPAN),
                "wt": wt_host,
                "wr": wr_host,
            }
        )
    return in_maps


def kernel(**inputs: np.ndarray) -> np.ndarray:
    global LAST_RESULTS
    x = np.ascontiguousarray(np.asarray(inputs["x"], dtype=np.float32))
    w = np.ascontiguousarray(np.asarray(inputs["w"], dtype=np.float32))
    assert x.shape == (B_TOTAL, F), x.shape
    assert w.shape == (C, F), w.shape

    from concourse.bass_utils import run_bass_kernel_spmd

    if "nc" not in _cache:
        nc = _build()
        _legalize_waits(nc)
        _cache["nc"] = nc
    nc = _cache["nc"]

    in_maps = _stage_inputs(x, w)
    res = run_bass_kernel_spmd(
        nc, in_maps, core_ids=list(range(N_CORES)),
        trace=bool(os.environ.get("BASS_TRACE")),
    )
    LAST_RESULTS = res
    return np.concatenate(
        [r["outT"].T.astype(np.float32) for r in res.results], axis=0
    )


if __name__ == "__main__":
    rng = np.random.default_rng(0)
    xs = rng.standard_normal((B_TOTAL, F), dtype=np.float32)
    ws = (rng.standard_normal((C, F)) * math.sqrt(2.0 / F)).astype(np.float32)
    o = kernel(x=xs, w=ws)
    print(o.shape, o.dtype, o[:2, :4])
